# revision 54
# baseline (speedup 1.0000x reference)
"""Trainium2 Bass kernel: GQA sliding-window attention with RoPE + attention sinks.

Problem: H=32 query heads, HKV=8 kv heads, D=128, S=2048, window=1024.
Sharding: 8 cores x (4 query heads + 1 kv head); each core runs full-sequence
banded attention for its head group; no cross-core communication.

Per-core algorithm (all matmuls fp16 with fp32 PSUM accumulation):
  - Inputs are cast to fp16 on the host during shard prep; output is fp16 on
    device, cast back to fp32 on the host (matches on-chip compute precision).
  - RoPE in natural [s, d] layout on DVE (rotate-half via free-dim APs),
    tables precomputed host-side as NEFF constants.
  - Q/K transposed to [d, s]: K + head-0 Q via tensor-engine identity matmuls
    (short latency during the pipeline fill), heads 1-3 via XBAR DMA
    transposes injected into the attention loop (keeps PE free for QK/PV).
  - Scores computed TRANSPOSED: psum[kj, qi] = kT.T @ qT, so exp(P^T) feeds
    the PV matmul directly as the stationary operand.
  - exp runs on ACT only -- the bottleneck engine (~58us busy of ~73us total);
    everything else is kept off it. Region 0 is bootstrapped in 2-tile rope
    spans interleaved with per-bank QK pieces + exps for the earliest start.
  - Causal/window masks: 0/1-triangle multiplies on DVE after exp.
  - No max-subtraction: logits ~ N(0,1) after the 1/sqrt(D) scale folded into
    the exp activation.
  - Softmax denominators from a ones-column appended to V tiles; attention
    sinks enter the denominator via a 1-col matmul in each PV chain.
  - Loads are need-ordered on SP/HWDGE (~650ns serialization per DMA op);
    quad-tile output stores go via the otherwise-idle gpsimd SWDGE path, with
    the last head's tail stores in pairs on SP to shorten the drain.
"""

import numpy as np

H, HKV, D, S = 32, 8, 128, 2048
NCORES = 8
HPC = H // NCORES          # query heads per core (4)
WINDOW = 1024
WTILES = WINDOW // 128     # 8
NT = S // 128              # 16 s-tiles
SM_SCALE = float(1.0 / np.sqrt(D))
PV_LAG = 13
VW = D + 1                 # v tile width with ones column

_CACHE = {}


def _region_width(t):
    return 128 * (min(t + WTILES, NT - 1) - t + 1)


def _build(repeat=1):
    import contextlib
    import concourse.mybir as mybir
    import concourse.tile as tile
    from concourse import bacc

    f32 = mybir.dt.float32
    f16 = mybir.dt.float16
    mult = mybir.AluOpType.mult
    add = mybir.AluOpType.add
    EXP = mybir.ActivationFunctionType.Exp

    nc = bacc.Bacc("TRN2", target_bir_lowering=False, debug=False,
                   num_devices=NCORES)

    q_ext = nc.declare_dram_parameter("q", [S, HPC * D], f16, isOutput=False)
    k_ext = nc.declare_dram_parameter("k", [S, D], f16, isOutput=False)
    v_ext = nc.declare_dram_parameter("v", [S, D], f16, isOutput=False)
    sink_ext = nc.declare_dram_parameter("sinks", [1, HPC], f32, isOutput=False)
    out_ext = nc.declare_dram_parameter("out", [S, HPC * D], f16, isOutput=True)

    # ---- host-precomputed constants (input-independent), one DRAM blob ----
    inv_freq = (1.0 / (10000.0 ** (np.arange(0, D, 2, dtype=np.float32) / D)))
    ang = np.arange(S, dtype=np.float32)[:, None] * inv_freq[None, :].astype(np.float32)
    cos = np.cos(ang).astype(np.float32)
    sin = np.sin(ang).astype(np.float32)
    cos_nat = np.concatenate([cos, cos], axis=1)          # [S, D]
    sinm_nat = np.concatenate([-sin, sin], axis=1)        # sign-folded rotate-half

    def to_tiles(tab):  # [S, D] -> [128, NT*D] with s-tile T at cols T*D
        return np.ascontiguousarray(
            tab.reshape(NT, 128, D).transpose(1, 0, 2).reshape(128, NT * D))

    jj = np.arange(128)[:, None]
    ii = np.arange(128)[None, :]
    mask_diag = np.where(jj <= ii, 1.0, 0.0).astype(np.float16)  # keep causal
    mask_win = np.where(jj > ii, 1.0, 0.0).astype(np.float16)    # keep window
    ones_blk = np.zeros((128, 128), np.float16)
    ones_blk[0, :] = 1.0

    cos_t = to_tiles(cos_nat).astype(np.float16)
    sin_t = to_tiles(sinm_nat).astype(np.float16)
    blocks = [cos_t[:, 0:512], sin_t[:, 0:512], np.eye(128, dtype=np.float16)]
    for c in range(1, 4):
        blocks += [cos_t[:, 512 * c:512 * (c + 1)],
                   sin_t[:, 512 * c:512 * (c + 1)]]
    blocks += [mask_diag, mask_win, ones_blk]
    consts = np.concatenate(blocks, axis=1)
    # per-chunk col offsets of cos/sinm inside the blob
    CC = [0, 1152, 2176, 3200]
    CS = [512, 1664, 2688, 3712]
    C_ID, C_MD, C_MW, C_ONE = 1024, 4224, 4352, 4480
    consts_dram = nc.inline_tensor(consts, "consts")

    offs = np.concatenate([[0], np.cumsum([_region_width(t) for t in range(NT)])])
    offs = [int(x) for x in offs]

    with tile.TileContext(nc) as tc:
        cst = nc.alloc_sbuf_tensor("cst", [128, consts.shape[1]], f16)
        kf = nc.alloc_sbuf_tensor("kf", [128, S], f16)       # k natural
        qf0 = nc.alloc_sbuf_tensor("qf0", [128, S], f16)     # q head0 natural
        qfx = nc.alloc_sbuf_tensor("qfx", [128, (HPC - 1) * S], f16)
        kT_sb = nc.alloc_sbuf_tensor("kT_sb", [128, S], f16)
        qT_sb = nc.alloc_sbuf_tensor("qT_sb", [128, HPC * S], f16)
        v1_sb = nc.alloc_sbuf_tensor("v1_sb", [128, NT * VW], f16)
        sink_raw = nc.alloc_sbuf_tensor("sink_raw", [1, HPC], f32)
        sink_exp = nc.alloc_sbuf_tensor("sink_exp", [1, HPC], f16)

        XW = (HPC - 1) * D  # per-row width of the heads-1..3 loads (384)

        with contextlib.ExitStack() as stk:
            qk_psum = stk.enter_context(
                tc.tile_pool(name="qk_psum", bufs=2, space="PSUM"))
            sp_psum = stk.enter_context(
                tc.tile_pool(name="sp_psum", bufs=2, space="PSUM"))
            rope_pool = stk.enter_context(tc.tile_pool(name="rope", bufs=6))
            roped_pool = stk.enter_context(tc.tile_pool(name="roped", bufs=8))
            norm_pool = stk.enter_context(tc.tile_pool(name="norm", bufs=6))
            ostage_pool = stk.enter_context(tc.tile_pool(name="ostage", bufs=6))
            expp_pool = stk.enter_context(tc.tile_pool(name="expp", bufs=3))

            # rope body: src3 is a [128, nt, 128] natural-layout AP covering
            # s-tiles [t0, t0+nt) (within one 4-tile table chunk) for one
            # head-stream; result lands transposed in dst_sb at dst_off,
            # either via PE identity matmuls (evac engine choice) or via an
            # XBAR DMA transpose on SP.
            def rope_span(src3, t0, nt, dst_sb, dst_off, via="xbar",
                          eng=None):
                eng = eng or nc.vector
                c, r = t0 // 4, 128 * (t0 % 4)
                w = 128 * nt
                cos3 = cst[:, CC[c] + r:CC[c] + r + w] \
                    .rearrange("p (T d) -> p T d", d=D)
                sin3 = cst[:, CS[c] + r:CS[c] + r + w] \
                    .rearrange("p (T d) -> p T d", d=D)
                tmp1 = rope_pool.tile([128, w], f16, tag="tmp1", name="tmp1")
                t13 = tmp1[:].rearrange("p (T d) -> p T d", d=D)
                eng.tensor_tensor(t13[:], src3[:], cos3[:], mult)
                tmp2 = rope_pool.tile([128, w], f16, tag="tmp2", name="tmp2")
                t23 = tmp2[:].rearrange("p (T d) -> p T d", d=D)
                # rotate-half as ONE op: negative-step AP swaps the 64-wide
                # halves; sinm is sign-folded per half already
                qsw = src3.rearrange("p T (x d) -> p T x d", d=64)[:, :, ::-1, :]
                s4 = sin3.rearrange("p T (x d) -> p T x d", d=64)
                t4 = t23.rearrange("p T (x d) -> p T x d", d=64)
                eng.tensor_tensor(t4, qsw, s4, mult)
                roped = roped_pool.tile([128, w], f16, tag="roped",
                                        name="roped")
                eng.tensor_tensor(roped[:], tmp1[:], tmp2[:], add)
                if via == "xbar":
                    nc.sync.dma_start(
                        out=dst_sb[:, dst_off:dst_off + w]
                        .rearrange("p (c j) -> p c j", j=128),
                        in_=roped[:], transpose=True)
                else:
                    ps = sp_psum.tile([128, w], f16, tag="sp", name="tp")
                    for a in range(nt):
                        nc.tensor.transpose(ps[:, 128 * a:128 * (a + 1)],
                                            roped[:, 128 * a:128 * (a + 1)],
                                            cst[:, C_ID:C_ID + 128])
                    if via == "pe_act":
                        nc.scalar.copy(dst_sb[:, dst_off:dst_off + w], ps[:])
                    else:  # "pe_dve"
                        nc.vector.tensor_copy(dst_sb[:, dst_off:dst_off + w],
                                              ps[:])

            def rope_k(c, via, t0=None, nt=4, eng=None):
                if t0 is None:
                    t0 = 4 * c
                rope_span(kf[:, 128 * t0:128 * (t0 + nt)]
                          .rearrange("p (T d) -> p T d", d=D),
                          t0, nt, kT_sb, 128 * t0, via, eng=eng)

            def rope_q0(t0, nt, via):
                rope_span(qf0[:, 128 * t0:128 * (t0 + nt)]
                          .rearrange("p (T d) -> p T d", d=D),
                          t0, nt, qT_sb, 128 * t0, via)

            def rope_q(h, c, via):
                if h == 0:
                    rope_q0(4 * c, 4, via)
                    return
                src3 = qfx[:].rearrange("p (T x) -> p T x", x=XW) \
                    [:, 4 * c:4 * (c + 1), D * (h - 1):D * h]
                rope_span(src3, 4 * c, 4, qT_sb, S * h + 512 * c, via)

            for _rep in range(repeat):
                # ---- input loads (SP HWDGE), need-ordered, 512-row grain ----
                def tbl_dma(a, n):
                    nc.sync.dma_start(out=cst[:, a:a + n],
                                      in_=consts_dram.ap()[:, a:a + n])

                def k_dma(t0, nt=4):
                    nc.sync.dma_start(
                        out=kf[:, 128 * t0:128 * (t0 + nt)]
                        .rearrange("p (T d) -> p T d", d=D),
                        in_=k_ext[128 * t0:128 * (t0 + nt), :]
                        .rearrange("(T p) d -> p T d", p=128))

                def q0_dma(t0, nt=4):
                    nc.sync.dma_start(
                        out=qf0[:, 128 * t0:128 * (t0 + nt)]
                        .rearrange("p (T d) -> p T d", d=D),
                        in_=q_ext[128 * t0:128 * (t0 + nt), 0:D]
                        .rearrange("(T p) d -> p T d", p=128))

                def qfx_dma(c):
                    nc.sync.dma_start(
                        out=qfx[:, 4 * XW * c:4 * XW * (c + 1)]
                        .rearrange("p (T x) -> p T x", x=XW),
                        in_=q_ext[512 * c:512 * (c + 1), D:HPC * D]
                        .rearrange("(T p) x -> p T x", p=128))

                v13 = v1_sb[:].rearrange("p (T w) -> p T w", w=VW)

                def v_dma(qtr):
                    nc.sync.dma_start(
                        out=v13[:, 4 * qtr:4 * (qtr + 1), 0:D],
                        in_=v_ext[512 * qtr:512 * (qtr + 1), :]
                        .rearrange("(T p) d -> p T d", p=128))

                # critical prefix on SP/HWDGE (each DMA costs ~650ns of HWDGE
                # serialization, so the prefix is kept short and merged)
                tbl_dma(0, 1152)          # cos0/sin0 + identity
                k_dma(0, 2)
                q0_dma(0, 2)
                k_dma(2, 2)
                q0_dma(2, 2)
                tbl_dma(1152, 1024)       # c1 tables
                q0_dma(4)
                k_dma(4)
                tbl_dma(2176, 1024)       # c2 tables
                q0_dma(8)
                tbl_dma(3200, 1024)       # c3 tables
                q0_dma(12)
                k_dma(8)
                tbl_dma(4224, 384)        # masks + ones column
                nc.sync.dma_start(out=sink_raw[:], in_=sink_ext[:])
                k_dma(12)
                qfx_dma(0)
                qfx_dma(1)
                v_dma(0)
                v_dma(1)
                qfx_dma(2)
                qfx_dma(3)
                v_dma(2)
                v_dma(3)

                # ones column for the softmax denominator (gpsimd; idle engine)
                nc.gpsimd.memset(v13[:, :, D:VW], 1.0)

                # rope schedule: bootstrap spans emitted inline with region-0
                # pieces below; the rest injected into the attention loop in
                # load-arrival order
                pending_ropes = [("k", 1), ("q0", 3), ("k", 2), ("k", 3)]

                # ---- attention: flat (head, key-tile-group) pipeline ----
                expPs = {}
                stages = {}

                def do_qk(h, group, front_split=False):
                    expP = expPs[h]
                    base = 0
                    regions = []
                    for t in group:
                        regions.append((t, base, _region_width(t)))
                        base += _region_width(t)
                    wtot = base
                    ps = qk_psum.tile([128, wtot], f32, tag="qk",
                                      name=f"qk{h}_{group[0]}")
                    # pieces split at PSUM bank boundaries; one start/stop per
                    # bank (front_split: stop+exp per bank for an early start)
                    bank_ops = {}
                    for t, rbase, w in regions:
                        cuts = {rbase, rbase + w}
                        for b in range(512, wtot, 512):
                            if rbase < b < rbase + w:
                                cuts.add(b)
                        cs = sorted(cuts)
                        for p0, p1 in zip(cs, cs[1:]):
                            bank_ops.setdefault(p0 // 512, []).append(
                                (t, rbase, p0, p1))
                    o0 = offs[group[0]]
                    for b, ops in sorted(bank_ops.items()):
                        for idx, (t, rbase, p0, p1) in enumerate(ops):
                            nc.tensor.matmul(
                                ps[:, p0:p1],
                                lhsT=kT_sb[:, 128 * t:128 * (t + 1)],
                                rhs=qT_sb[:, S * h + 128 * t + (p0 - rbase):
                                          S * h + 128 * t + (p1 - rbase)],
                                start=(idx == 0),
                                stop=(idx == len(ops) - 1))
                        if front_split:
                            lo, hi = 512 * b, min(512 * (b + 1), wtot)
                            nc.scalar.activation(expP[:, o0 + lo:o0 + hi],
                                                 ps[:, lo:hi], EXP,
                                                 scale=SM_SCALE)
                    if not front_split:
                        nc.scalar.activation(expP[:, o0:o0 + wtot],
                                             ps[:, 0:wtot], EXP, scale=SM_SCALE)
                    # causal/window masks: 0/1-triangle multiplies on DVE; when
                    # both apply (w=1152) one op covers blocks 0 and 8 via a
                    # strided AP -- except when front_split (separate stops)
                    for t, rbase, w in regions:
                        o = offs[t]
                        if t + WTILES <= NT - 1 and not front_split:
                            blk = expP[:, o:o + w] \
                                .rearrange("p (a b) -> p a b", b=128)[:, 0:9:8, :]
                            msk = cst[:, C_MD:C_MD + 256] \
                                .rearrange("p (a b) -> p a b", b=128)
                            nc.vector.tensor_tensor(blk, blk, msk, mult)
                        else:
                            nc.vector.tensor_tensor(
                                expP[:, o:o + 128], expP[:, o:o + 128],
                                cst[:, C_MD:C_MD + 128], mult)
                            if t + WTILES <= NT - 1:
                                wo = o + 128 * WTILES
                                nc.vector.tensor_tensor(
                                    expP[:, wo:wo + 128], expP[:, wo:wo + 128],
                                    cst[:, C_MW:C_MW + 128], mult)

                pv_idx = [0]

                def do_pv(h, qt):
                    expP = expPs[h]
                    quad = qt // 4
                    if qt % 4 == 0:
                        stages[(h, quad)] = ostage_pool.tile(
                            [128, 4 * D], f16, tag="ost", name=f"ost{h}_{quad}")
                    stage = stages[(h, quad)]
                    t_lo = max(0, qt - WTILES)
                    idx = pv_idx[0]
                    pv_idx[0] += 1
                    # final drain: the qk psum pool is retired by then -- borrow
                    # its banks as extra PV slots to double drain throughput
                    if idx >= HPC * NT - 6 and idx % 2 == 1:
                        po = sp_psum.tile([128, VW], f32, tag="sp",
                                          name=f"pv{h}_{qt}")
                    elif idx >= HPC * NT - 6:
                        po = qk_psum.tile([128, VW], f32, tag="qk",
                                          name=f"pv{h}_{qt}")
                    else:
                        po = sp_psum.tile([128, VW], f32, tag="sp",
                                          name=f"pv{h}_{qt}")
                    # sink term right after the chain-opening matmul so the
                    # normalize isn't gated on a trailing matmul
                    single = qt == t_lo
                    for t in range(t_lo, qt + 1):
                        nc.tensor.matmul(
                            po[:],
                            lhsT=expP[:, offs[t] + 128 * (qt - t):
                                      offs[t] + 128 * (qt - t) + 128],
                            rhs=v1_sb[:, t * VW:(t + 1) * VW],
                            start=(t == t_lo),
                            stop=(t == qt) and not single)
                        if t == t_lo:
                            nc.tensor.matmul(po[:, D:D + 1],
                                             lhsT=cst[0:1, C_ONE:C_ONE + 128],
                                             rhs=sink_exp[0:1, h:h + 1],
                                             start=False, stop=single)
                    recip = norm_pool.tile([128, 1], f32, tag="recip",
                                           name="recip")
                    nc.vector.reciprocal(recip[:], po[:, D:D + 1])
                    j = qt % 4
                    nc.vector.tensor_scalar(stage[:, D * j:D * (j + 1)],
                                            po[:, 0:D], recip[:], None, mult)
                    # last head: store in pairs (and via SP at the very end)
                    # so the final store's latency chain is short
                    if h == HPC - 1 and qt >= 12:
                        if qt >= 14:
                            nc.sync.dma_start(
                                out=out_ext[128 * qt:128 * (qt + 1),
                                            D * h:D * (h + 1)]
                                .rearrange("(T p) d -> p T d", p=128),
                                in_=stage[:, D * j:D * (j + 1)]
                                .rearrange("p (T d) -> p T d", d=D))
                        elif j % 2 == 1:
                            nc.sync.dma_start(
                                out=out_ext[256 * (qt // 2):256 * (qt // 2 + 1),
                                            D * h:D * (h + 1)]
                                .rearrange("(T p) d -> p T d", p=128),
                                in_=stage[:, D * (j - 1):D * (j + 1)]
                                .rearrange("p (T d) -> p T d", d=D))
                    elif j == 3:
                        nc.gpsimd.dma_start(
                            out=out_ext[512 * quad:512 * (quad + 1),
                                        D * h:D * (h + 1)]
                            .rearrange("(T p) d -> p T d", p=128),
                            in_=stage[:].rearrange("p (T d) -> p T d", d=D))

                pending = []
                done = 0

                # ---- region-0 bootstrap: 2-tile rope spans interleaved with
                # QK/exp pieces so the first exp fires as early as possible
                expPs[0] = expp_pool.tile([128, offs[NT]], f16, tag="expp",
                                          name="expP0")
                e0 = expPs[0]
                ps0 = qk_psum.tile([128, 1152], f32, tag="qk", name="qk0_0")

                def qk0_piece(p0, p1):
                    nc.tensor.matmul(ps0[:, p0:p1], lhsT=kT_sb[:, 0:128],
                                     rhs=qT_sb[:, p0:p1], start=True, stop=True)
                    nc.scalar.activation(e0[:, p0:p1], ps0[:, p0:p1], EXP,
                                         scale=SM_SCALE)

                rope_k(None, "pe_act", t0=0, nt=2)
                rope_q0(0, 2, "pe_act")
                qk0_piece(0, 256)
                nc.vector.tensor_tensor(e0[:, 0:128], e0[:, 0:128],
                                        cst[:, C_MD:C_MD + 128], mult)
                rope_q0(2, 2, "pe_act")
                qk0_piece(256, 512)
                rope_k(None, "pe_dve", t0=2, nt=2)
                rope_q(0, 1, "pe_dve")
                qk0_piece(512, 1024)
                rope_q(0, 2, "pe_dve")
                qk0_piece(1024, 1152)
                nc.vector.tensor_tensor(e0[:, 1024:1152], e0[:, 1024:1152],
                                        cst[:, C_MW:C_MW + 128], mult)
                # sink prep on ACT, after the bootstrap exps so the ACT queue
                # isn't head-blocked on the sinks DMA
                nc.scalar.activation(sink_exp[:], sink_raw[:], EXP)
                pending.append((0, 0))

                GROUPS = [[t] for t in range(10)] + [[10, 11], [12, 13, 14, 15]]
                GROUPS_LAST = [[t] for t in range(10)] + [[10, 11], [12, 13],
                                                          [14, 15]]
                HG = [GROUPS_LAST if h == HPC - 1 else GROUPS
                      for h in range(HPC)]
                steps = [(h, gi) for h in range(HPC)
                         for gi in range(len(HG[h])) if (h, gi) != (0, 0)]
                # rope/XBAR injection: head-0 leftovers in load-arrival order,
                # then the next head's q chunks
                INJECT = {5: 0, 6: 1, 7: 2, 8: 3}
                pending = pending
                for i, (h, gi) in enumerate(steps):
                    if gi == 0:
                        expPs[h] = expp_pool.tile([128, offs[NT]], f16,
                                                  tag="expp", name=f"expP{h}")
                    if h == 0 and gi in (1, 2, 3, 4) and pending_ropes:
                        kind, c = pending_ropes.pop(0)
                        if kind == "k":
                            rope_k(c, "pe_dve")
                        else:
                            rope_q(0, c, "pe_dve")
                    if gi in INJECT and h + 1 < HPC:
                        rope_q(h + 1, INJECT[gi], "xbar")
                    do_qk(h, HG[h][gi])
                    for t in HG[h][gi]:
                        pending.append((h, t))
                    lag_eff = min(PV_LAG, max(1, len(steps) - 1 - i))
                    while done < len(pending) - lag_eff:
                        do_pv(*pending[done])
                        done += 1
                while done < len(pending):
                    do_pv(*pending[done])
                    done += 1

    nc.compile()
    return nc


def _get_nc(repeat=1):
    key = f"nc{repeat}"
    if key not in _CACHE:
        _CACHE[key] = _build(repeat)
    return _CACHE[key]


def kernel(q, k, v, attention_sinks, attention_window_size=1024):
    from concourse.bass_utils import run_bass_kernel_spmd

    assert int(attention_window_size) == WINDOW, "kernel compiled for window=1024"
    q = np.asarray(q, dtype=np.float16)
    k = np.asarray(k, dtype=np.float16)
    v = np.asarray(v, dtype=np.float16)
    sinks = np.asarray(attention_sinks, dtype=np.float32).reshape(H)

    nc = _get_nc()
    in_maps = []
    for c in range(NCORES):
        in_maps.append({
            "q": np.ascontiguousarray(q[:, c * HPC * D:(c + 1) * HPC * D]),
            "k": np.ascontiguousarray(k[:, c * D:(c + 1) * D]),
            "v": np.ascontiguousarray(v[:, c * D:(c + 1) * D]),
            "sinks": np.ascontiguousarray(sinks[c * HPC:(c + 1) * HPC]
                                          .reshape(1, HPC)),
        })
    res = run_bass_kernel_spmd(nc, in_maps, core_ids=list(range(NCORES)))
    out = np.empty((S, H * D), dtype=np.float32)
    for c in range(NCORES):
        out[:, c * HPC * D:(c + 1) * HPC * D] = \
            res.results[c]["out"].astype(np.float32)
    return out
